# revision 4
# baseline (speedup 1.0000x reference)
"""Trainium2 Bass kernel for nn_CG_model (GNN message passing).

Strategy (SPMD on 8 NeuronCores):
  - Host: node-level monotonic net (E_mlp), per-edge feature gathers,
    W-net (1-D in r), final per-edge combine + segment-sum (cheap, O(E)).
  - Device (per core, E/8 edges): the dominant compute — the A/B/C
    2->32->32->1 SiLU MLPs evaluated at (r, T_i), (r, T_j) plus their
    eps-shifted finite-difference companions, restructured as
    Δ-propagation so reduced-precision (float32r) matmuls keep the FD
    differences accurate.  Outputs 12 channels/edge:
    [A_i, A_j, DA_i, DA_j, B..., C...] with DA = (A(T+eps)-A(T))/eps.
"""
import math
import numpy as np

import concourse.bass as bass
import concourse.mybir as mybir
import concourse.tile as tile
from concourse.bass_utils import run_bass_kernel_spmd

F32 = mybir.dt.float32
F32R = mybir.dt.float32r
AF = mybir.ActivationFunctionType

P = 128
B = 512            # edges per batch
N_CORES = 8
E_TOTAL = 800000
E_SHARD = 100352   # ceil(E/8 /1024)*1024 -> 196 batches of 512
N_BATCH = E_SHARD // B
XCHUNK = 8         # batches per X1 chunk load
D = 3
H = 1.0
DT = 0.01
EPS_T = 1e-3


# ----------------------------------------------------------------- device ---
def _legalize_waits(nc):
    """Old-walrus compat: hoist excess sem waits into standalone EVSEMs."""
    cnt = [0]
    for f in nc.m.functions:
        for b in f.blocks:
            insts = b.instructions
            out = []
            changed = False
            for inst in insts:
                si = inst.sync_info
                waits = list(si.on_wait) if (si is not None and si.on_wait) else []
                cap = 2 if isinstance(inst, mybir.InstEventSemaphore) else 1
                if len(waits) > cap:
                    changed = True
                    keep = waits[-cap:]
                    extra = waits[:-cap]
                    for k in range(0, len(extra), 2):
                        cnt[0] += 1
                        out.append(mybir.InstEventSemaphore(
                            name=f"I-waitsplit-{cnt[0]}",
                            engine=inst.engine, ins=[], outs=[],
                            sync_info=mybir.SyncInfo(on_wait=extra[k:k+2], on_update=[]),
                        ))
                    inst.sync_info = mybir.SyncInfo(on_wait=keep, on_update=list(si.on_update))
                out.append(inst)
            if changed:
                insts[:] = out
    return cnt[0]


def build_program(act=AF.Silu, legalize=True):
    nc = bass.Bass()
    x1 = nc.declare_dram_parameter("x1", [3, E_SHARD], F32, isOutput=False)
    wl1 = nc.declare_dram_parameter("wl1", [3, 3, 64], F32, isOutput=False)
    b1n = nc.declare_dram_parameter("b1n", [3, 64, 1], F32, isOutput=False)
    b1ne = nc.declare_dram_parameter("b1ne", [3, 64, 1], F32, isOutput=False)
    wl2 = nc.declare_dram_parameter("wl2", [3, 128, 128], F32, isOutput=False)
    b2r = nc.declare_dram_parameter("b2r", [3, 64, 1], F32, isOutput=False)
    wl3 = nc.declare_dram_parameter("wl3", [3, 128, 12], F32, isOutput=False)
    out = nc.declare_dram_parameter("out", [12, E_SHARD], F32, isOutput=True)

    with tile.TileContext(nc) as tc:
        with (
            tc.tile_pool(name="const", bufs=1) as cp,
            tc.tile_pool(name="sb", bufs=2) as sb,
            tc.tile_pool(name="xc", bufs=2) as xc,
            tc.tile_pool(name="ps", bufs=2, space="PSUM") as ps,
            tc.tile_pool(name="ps2", bufs=3, space="PSUM") as ps2,
        ):
            # constants (load once, f32r via SWDGE cast where matmul operand)
            w1_t = cp.tile([3, 3, 64], F32R, name="w1_t")
            nc.gpsimd.dma_start(out=w1_t[:], in_=wl1[:].rearrange("n k m -> k n m"))
            w2_t = cp.tile([128, 3, 128], F32R, name="w2_t")
            nc.gpsimd.dma_start(out=w2_t[:], in_=wl2[:].rearrange("n k m -> k n m"))
            w3_t = cp.tile([128, 3, 12], F32R, name="w3_t")
            nc.gpsimd.dma_start(out=w3_t[:], in_=wl3[:].rearrange("n k m -> k n m"))
            b1n_t = cp.tile([64, 3, 1], F32, name="b1n_t")
            nc.sync.dma_start(out=b1n_t[:], in_=b1n[:].rearrange("n k o -> k n o"))
            b1ne_t = cp.tile([64, 3, 1], F32, name="b1ne_t")
            nc.sync.dma_start(out=b1ne_t[:], in_=b1ne[:].rearrange("n k o -> k n o"))
            b2r_t = cp.tile([64, 3, 1], F32, name="b2r_t")
            nc.sync.dma_start(out=b2r_t[:], in_=b2r[:].rearrange("n k o -> k n o"))

            for bi in range(N_BATCH):
                if bi % XCHUNK == 0:
                    xck = xc.tile([3, XCHUNK * B], F32R, name="xck", tag="xck")
                    lo = bi * B
                    hi = min(E_SHARD, lo + XCHUNK * B)
                    nc.gpsimd.dma_start(out=xck[:, : hi - lo], in_=x1[:, lo:hi])
                xs = xck[:, (bi % XCHUNK) * B : (bi % XCHUNK) * B + B]

                # ---- L1 (per net, h1 at partitions 0-63) + L2 sources
                # L2src layout per net: rows 0-63 = deltas (aligned sub),
                # rows 64-127 = bases (partition-shifted DVE copy).
                l2srcs = []
                for g in range(3):
                    z1 = ps.tile([64, B], F32, space="PSUM", name=f"z1_{g}", tag="z1")
                    nc.tensor.matmul(out=z1[:], lhsT=w1_t[:, g, :], rhs=xs,
                                     start=True, stop=True)
                    h1 = sb.tile([64, B], F32, name=f"h1_{g}", tag="h1")
                    nc.scalar.activation(h1[:], z1[:], act, bias=b1n_t[:, g, :])
                    h1e = sb.tile([64, B], F32, name=f"h1e_{g}", tag="h1e")
                    nc.scalar.activation(h1e[:], z1[:], act, bias=b1ne_t[:, g, :])
                    l2s = sb.tile([128, B], F32R, name=f"l2s_{g}", tag="l2s")
                    nc.vector.tensor_tensor(out=l2s[0:64, :], in0=h1e[:],
                                            in1=h1[:], op=mybir.AluOpType.subtract)
                    nc.vector.tensor_copy(out=l2s[64:128, :], in_=h1[:])
                    l2srcs.append(l2s)

                l3srcs = []
                for g, l2s in enumerate(l2srcs):
                    z2 = ps2.tile([128, B], F32, space="PSUM", name=f"z2_{g}", tag="z2")
                    nc.tensor.matmul(out=z2[:], lhsT=w2_t[:, g, :], rhs=l2s[:],
                                     start=True, stop=True)
                    # z2 layout: rows 0-63 = bases (z2_i, z2_j), 64-127 = dz2
                    dz2 = sb.tile([64, B], F32, name=f"dz2_{g}", tag="dz2")
                    nc.vector.tensor_copy(out=dz2[:], in_=z2[64:128, :])
                    u2 = sb.tile([64, B], F32, name=f"u2_{g}", tag="u2")
                    nc.vector.tensor_tensor(out=u2[:], in0=z2[0:64, :],
                                            in1=dz2[:], op=mybir.AluOpType.add)
                    h2 = sb.tile([64, B], F32, name=f"h2_{g}", tag="h2")
                    nc.scalar.activation(h2[:], z2[0:64, :], act, bias=b2r_t[:, g, :])
                    h2e = sb.tile([64, B], F32, name=f"h2e_{g}", tag="h2e")
                    nc.scalar.activation(h2e[:], u2[:], act, bias=b2r_t[:, g, :])
                    l3s = sb.tile([128, B], F32R, name=f"l3s_{g}", tag="l3s")
                    nc.vector.tensor_tensor(out=l3s[0:64, :], in0=h2e[:],
                                            in1=h2[:], op=mybir.AluOpType.subtract)
                    nc.vector.tensor_copy(out=l3s[64:128, :], in_=h2[:])
                    l3srcs.append(l3s)

                l3o = ps.tile([12, B], F32, space="PSUM", name="l3o", tag="l3o")
                for g, l3s in enumerate(l3srcs):
                    nc.tensor.matmul(out=l3o[:], lhsT=w3_t[:, g, :], rhs=l3s[:],
                                     start=(g == 0), stop=(g == 2))
                ev = sb.tile([12, B], F32, name="ev", tag="ev")
                nc.vector.tensor_copy(out=ev[:], in_=l3o[:])
                nc.sync.dma_start(out=out[:, bi * B : bi * B + B], in_=ev[:])

    if legalize:
        _legalize_waits(nc)
    return nc


# ------------------------------------------------------------------- host ---
def _silu(x):
    return x / (1.0 + np.exp(-x))


def _softplus(x):
    return np.logaddexp(np.float32(0.0), x).astype(np.float32)


def _np_params(params):
    out = {}
    for k in ("W_mlp", "E_mlp", "A_mlp", "B_mlp", "C_mlp"):
        out[k] = [(np.asarray(W, np.float32), np.asarray(b, np.float32))
                  for W, b in params[k]]
    out["log_k_B"] = float(np.asarray(params["log_k_B"]))
    out["log_m"] = float(np.asarray(params["log_m"]))
    return out


def _host_nodes(S, d, params):
    T_SIGNS = np.array([1.0, -1.0], np.float32)
    N = S.shape[0]
    Vn = (1.0 / d).astype(np.float32)
    EPS = np.float32(0.01)
    S_pert = np.concatenate([S, S + EPS, S, S - EPS], 0)
    V_pert = np.concatenate([Vn, Vn, Vn + EPS, Vn], 0)
    x = np.concatenate([S_pert, V_pert], -1)
    layers = params["E_mlp"]
    n = len(layers)
    for idx, (W, b) in enumerate(layers):
        Wa = np.abs(W)
        if idx == 0:
            Wa = Wa * T_SIGNS
        y = x @ Wa.T + b
        x = _softplus(y) if idx < n - 1 else y
    U, U_Sp, U_Vp, U_Sm = x[:N], x[N:2*N], x[2*N:3*N], x[3*N:]
    T = (U_Sp - U) / EPS
    Pn = -(U_Vp - U) / EPS
    C = T * EPS**2 / (U_Sp - 2*U + U_Sm)
    return U, T, Pn, C


def _wnet_host(layers, r, eps):
    """Exact W-net FD on host: Wk(r±eps), dW_dr."""
    (W1, b1), (W2, b2), (W3, b3) = layers

    def mlp(xr):
        h = _silu(np.outer(xr, W1[:, 0]) + b1)
        h = _silu(h @ W2.T + b2)
        return h @ W3.T + b3

    sm = (r - eps) / H
    sp = (r + eps) / H
    om = mlp(sm)[:, 0]
    op = mlp(sp)[:, 0]
    Wk_m = np.exp(om) * (1 - sm**2)
    Wk_p = np.exp(op) * (1 - sp**2)
    return (Wk_p - Wk_m) / (2 * eps * r)


def _pack_weights(params):
    """Device constant arrays. K-layouts: deltas rows 0-63, bases 64-127."""
    packs = {}
    nets = [params["A_mlp"], params["B_mlp"], params["C_mlp"]]
    wl1 = np.zeros((3, 3, 64), np.float32)
    b1n = np.zeros((3, 64, 1), np.float32)
    b1ne = np.zeros((3, 64, 1), np.float32)
    for g, net in enumerate(nets):
        (W1, b1), _, _ = net
        wl1[g, 0, 0:32] = W1[:, 0] / H
        wl1[g, 1, 0:32] = W1[:, 1]
        wl1[g, 0, 32:64] = W1[:, 0] / H
        wl1[g, 2, 32:64] = W1[:, 1]
        b1n[g, :, 0] = np.concatenate([b1, b1])
        shift = EPS_T * W1[:, 1]
        b1ne[g, :, 0] = np.concatenate([b1 + shift, b1 + shift])

    # L2 lhsT [K=128, M=128]: K rows 0-63 = d1(i,j), 64-127 = h1(i,j);
    # M cols: [z2_i, z2_j, dz2_i, dz2_j]
    wl2 = np.zeros((3, 128, 128), np.float32)
    b2r = np.zeros((3, 64, 1), np.float32)
    for g, net in enumerate(nets):
        _, (W2, b2), _ = net
        wl2[g, 64:96, 0:32] = W2.T       # z2_i <- h1_i
        wl2[g, 96:128, 32:64] = W2.T     # z2_j <- h1_j
        wl2[g, 0:32, 64:96] = W2.T       # dz2_i <- d1_i
        wl2[g, 32:64, 96:128] = W2.T     # dz2_j <- d1_j
        b2r[g, :, 0] = np.concatenate([b2, b2])

    # L3 lhsT [K=128, M=12]: K rows 0-63 = d2(i,j), 64-127 = h2(i,j)
    wl3 = np.zeros((3, 128, 12), np.float32)
    for g, net in enumerate(nets):
        _, _, (W3, b3) = net
        w3 = W3[0]
        c = 4 * g
        wl3[g, 64:96, c + 0] = w3             # A_i
        wl3[g, 96:128, c + 1] = w3            # A_j
        wl3[g, 0:32, c + 2] = w3 / EPS_T      # DA_i
        wl3[g, 32:64, c + 3] = w3 / EPS_T     # DA_j
    packs.update(wl1=wl1, b1n=b1n, b1ne=b1ne, wl2=wl2, b2r=b2r, wl3=wl3)
    return packs


_CACHED = {}


def kernel(v, edge_index, r_ij, S, d, dW, dV, params):
    v = np.asarray(v, np.float32)
    edge_index = np.asarray(edge_index)
    r_ij = np.asarray(r_ij, np.float32)
    S = np.asarray(S, np.float32)
    d_ = np.asarray(d, np.float32)
    dW = np.asarray(dW, np.float32)
    dV = np.asarray(dV, np.float32)
    params = _np_params(params)
    N = v.shape[0]
    E = r_ij.shape[0]
    i_idx = np.asarray(edge_index[0]).astype(np.int64)
    j_idx = np.asarray(edge_index[1]).astype(np.int64)

    k_B = np.float32(math.exp(params["log_k_B"]))
    m = np.float32(math.exp(params["log_m"]))
    sdt = np.float32(1.0 / math.sqrt(DT))
    sq = np.float32(math.sqrt(2 * k_B) * sdt)

    U, T, Pn, C = _host_nodes(S, d_, params)
    T1, P1, C1 = T[:, 0], Pn[:, 0], C[:, 0]

    r_norm = np.sqrt((r_ij ** 2).sum(-1)).astype(np.float32)
    t_i = T1[i_idx].astype(np.float32)
    t_j = T1[j_idx].astype(np.float32)

    # ---- device: 12 MLP channels per edge
    if "nc" not in _CACHED:
        _CACHED["nc"] = build_program()
    nc = _CACHED["nc"]
    packs = _pack_weights(params)

    ES = E_SHARD
    in_maps = []
    for c in range(N_CORES):
        lo = c * (E // N_CORES)
        hi = (c + 1) * (E // N_CORES)
        x1 = np.zeros((3, ES), np.float32)
        n = hi - lo
        x1[0, :n] = r_norm[lo:hi]
        x1[1, :n] = t_i[lo:hi]
        x1[2, :n] = t_j[lo:hi]
        if n < ES:  # pad with edge 0 copies (harmless values)
            x1[0, n:] = r_norm[lo]
            x1[1, n:] = t_i[lo]
            x1[2, n:] = t_j[lo]
        im = dict(packs)
        im["x1"] = x1
        in_maps.append(im)

    res = run_bass_kernel_spmd(nc, in_maps, list(range(N_CORES)))
    outs = [res.results[c]["out"] for c in range(N_CORES)]
    mlp = np.concatenate([o[:, : E // N_CORES] for o in outs], axis=1)  # [12, E]

    A_i, A_j, DA_i, DA_j = mlp[0], mlp[1], mlp[2], mlp[3]
    B_i, B_j, DB_i, DB_j = mlp[4], mlp[5], mlp[6], mlp[7]
    Cc_i, Cc_j, DC_i, DC_j = mlp[8], mlp[9], mlp[10], mlp[11]
    b3A = np.float32(params["A_mlp"][2][1][0])
    b3B = np.float32(params["B_mlp"][2][1][0])
    b3C = np.float32(params["C_mlp"][2][1][0])
    A_i = A_i + b3A; A_j = A_j + b3A
    B_i = B_i + b3B; B_j = B_j + b3B
    Cc_i = Cc_i + b3C; Cc_j = Cc_j + b3C

    # ---- host: per-edge channels
    e_ij = r_ij / (r_norm[:, None] + 1e-8)
    v_ij = v[i_idx] - v[j_idx]
    ev = (e_ij * v_ij).sum(-1).astype(np.float32)
    vv = (v_ij * v_ij).sum(-1).astype(np.float32)
    tr = np.trace(dW, axis1=1, axis2=2).astype(np.float32)
    dW_bar = (0.5 * (dW + np.swapaxes(dW, 1, 2))
              - np.eye(3, dtype=np.float32)[None] * tr[:, None, None] / D)
    dWe = np.einsum('eab,eb->ea', dW_bar, e_ij).astype(np.float32)
    Pd = (P1[i_idx] / d_[i_idx, 0] ** 2 + P1[j_idx] / d_[j_idx, 0] ** 2).astype(np.float32)
    u_i = (1.0 / (C1[i_idx] * T1[i_idx])).astype(np.float32)
    u_j = (1.0 / (C1[j_idx] * T1[j_idx])).astype(np.float32)
    ci = (1.0 / C1[i_idx]).astype(np.float32)
    cj = (1.0 / C1[j_idx]).astype(np.float32)
    Ts = (1.0 / T1[i_idx] + 1.0 / T1[j_idx]).astype(np.float32)
    Td = (1.0 / T1[i_idx] - 1.0 / T1[j_idx]).astype(np.float32)
    dv = dV[:, 0]

    dW_dr = _wnet_host(params["W_mlp"], r_norm, EPS_T)
    grad_W = dW_dr[:, None] * (e_ij * (r_norm[:, None] + 1e-8))
    termPd = Pd[:, None] * grad_W

    A_ij = A_i * A_j; B_ij = B_i * B_j; C_ij = Cc_i * Cc_j
    gA_i = 2 * A_ij * A_j * DA_i; gA_j = 2 * A_ij * A_i * DA_j
    gB_i = 2 * B_ij * B_j * DB_i; gB_j = 2 * B_ij * B_i * DB_j
    gC_i = 2 * C_ij * Cc_j * DC_i; gC_j = 2 * C_ij * Cc_i * DC_j
    A2 = A_ij ** 2; B2 = B_ij ** 2
    auxMSV = 0.5 * A2[:, None] * v_ij + (0.5 * A2 + (B2 - A2) / D)[:, None] * ev[:, None] * e_ij
    termMSV = Ts[:, None] * auxMSV
    tv = -(u_i + u_j)[:, None] * auxMSV
    gi_v = ((gA_i / 2)[:, None] * v_ij
            + (gA_i / 2 + (gB_i - gA_i) / D)[:, None] * ev[:, None] * e_ij) * ci[:, None]
    gj_v = ((gA_j / 2)[:, None] * v_ij
            + (gA_j / 2 + (gB_j - gA_j) / D)[:, None] * ev[:, None] * e_ij) * cj[:, None]
    q = tv + gi_v + gj_v
    noise = A_ij[:, None] * dWe + (B_ij * tr / D)[:, None] * e_ij
    alpha = (-termPd - 0.5 * termMSV - 0.5 * k_B * q) / m + (np.sqrt(2 * k_B) / m * sdt) * noise
    tn = -0.5 * (noise * v_ij).sum(-1)
    tC = C_ij * dv
    auxMSS = (A2 / 2 * vv + (A2 / 2 + (B2 - A2) / D) * ev ** 2) / 4
    C2s = C_ij ** 2
    mS_i = Ts * auxMSS + Td * C2s
    mS_j = Ts * auxMSS - Td * C2s
    s1_i = -(2 * u_i + u_j) * auxMSS
    s1_j = -(2 * u_j + u_i) * auxMSS
    hi = (gA_i / 2 * vv + (gA_i / 2 + (gB_i - gA_i) / D) * ev ** 2) * ci / 4
    hj = (gA_j / 2 * vv + (gA_j / 2 + (gB_j - gA_j) / D) * ev ** 2) * cj / 4
    h_mix = (gA_i / 2 * vv + (gA_j / 2 + (gB_i - gA_i) / D) * ev ** 2) * ci / 4
    s4_i = -(2 * u_i - u_j) * C2s
    s4_j = -(2 * u_j - u_i) * C2s
    p5 = gC_i * ci - gC_j * cj
    tmv = -((D + 1) * A2 / 2 + (B2 - A2) / D)
    b_i = mS_i + k_B * (s1_i + hi + hj + s4_i + p5 + tmv / m) + sq * (tn + tC)
    b_j = mS_j + k_B * (s1_j + hj + h_mix + s4_j - p5 + tmv / m) + sq * (tn - tC)

    # ---- segment sum (host)
    ACC = np.zeros((N, 4), np.float32)
    for c_ in range(3):
        ACC[:, c_] += np.bincount(i_idx, weights=alpha[:, c_], minlength=N).astype(np.float32)
        ACC[:, c_] -= np.bincount(j_idx, weights=alpha[:, c_], minlength=N).astype(np.float32)
    ACC[:, 3] += np.bincount(i_idx, weights=b_i, minlength=N).astype(np.float32)
    ACC[:, 3] += np.bincount(j_idx, weights=b_j, minlength=N).astype(np.float32)

    dvdt = ACC[:, 0:3]
    dSdt = (ACC[:, 3] / T1)[:, None].astype(np.float32)
    E_out = (U + 0.5 * m * (v * v).sum(-1, keepdims=True)).astype(np.float32)
    return dvdt, dSdt, E_out


# revision 6
# speedup vs baseline: 1.2033x; 1.2033x over previous
"""Trainium2 Bass kernel for nn_CG_model (GNN message passing).

Strategy (SPMD on 8 NeuronCores):
  - Host: node-level monotonic net (E_mlp), per-edge feature gathers,
    W-net (1-D in r), final per-edge combine + segment-sum (cheap, O(E)).
  - Device (per core, E/8 edges): the dominant compute — the A/B/C
    2->32->32->1 SiLU MLPs evaluated at (r, T_i), (r, T_j) plus their
    eps-shifted finite-difference companions, restructured as
    Δ-propagation so reduced-precision (float32r) matmuls keep the FD
    differences accurate.  Outputs 12 channels/edge:
    [A_i, A_j, DA_i, DA_j, B..., C...] with DA = (A(T+eps)-A(T))/eps.
"""
import math
import numpy as np

import concourse.bass as bass
import concourse.mybir as mybir
import concourse.tile as tile
from concourse.bass_utils import run_bass_kernel_spmd

F32 = mybir.dt.float32
F32R = mybir.dt.float32r
AF = mybir.ActivationFunctionType

P = 128
B = 512            # edges per batch
N_CORES = 8
E_TOTAL = 800000
E_SHARD = 100352   # ceil(E/8 /1024)*1024 -> 196 batches of 512
N_BATCH = E_SHARD // B
XCHUNK = 8         # batches per X1 chunk load
D = 3
H = 1.0
DT = 0.01
EPS_T = 1e-3


# ----------------------------------------------------------------- device ---
def _legalize_waits(nc):
    """Old-walrus compat: hoist excess sem waits into standalone EVSEMs."""
    cnt = [0]
    for f in nc.m.functions:
        for b in f.blocks:
            insts = b.instructions
            out = []
            changed = False
            for inst in insts:
                si = inst.sync_info
                waits = list(si.on_wait) if (si is not None and si.on_wait) else []
                cap = 2 if isinstance(inst, mybir.InstEventSemaphore) else 1
                if len(waits) > cap:
                    changed = True
                    keep = waits[-cap:]
                    extra = waits[:-cap]
                    for k in range(0, len(extra), 2):
                        cnt[0] += 1
                        out.append(mybir.InstEventSemaphore(
                            name=f"I-waitsplit-{cnt[0]}",
                            engine=inst.engine, ins=[], outs=[],
                            sync_info=mybir.SyncInfo(on_wait=extra[k:k+2], on_update=[]),
                        ))
                    inst.sync_info = mybir.SyncInfo(on_wait=keep, on_update=list(si.on_update))
                out.append(inst)
            if changed:
                insts[:] = out
    return cnt[0]


def build_program(act=AF.Silu, legalize=True):
    nc = bass.Bass()
    x1 = nc.declare_dram_parameter("x1", [3, E_SHARD], F32, isOutput=False)
    wl1 = nc.declare_dram_parameter("wl1", [3, 3, 64], F32, isOutput=False)
    wl1ab = nc.declare_dram_parameter("wl1ab", [2, 3, 128], F32, isOutput=False)
    b1ab = nc.declare_dram_parameter("b1ab", [128, 1], F32, isOutput=False)
    b1abe = nc.declare_dram_parameter("b1abe", [128, 1], F32, isOutput=False)
    b1n = nc.declare_dram_parameter("b1n", [3, 64, 1], F32, isOutput=False)
    b1ne = nc.declare_dram_parameter("b1ne", [3, 64, 1], F32, isOutput=False)
    wl2 = nc.declare_dram_parameter("wl2", [3, 128, 128], F32, isOutput=False)
    b2r = nc.declare_dram_parameter("b2r", [3, 64, 1], F32, isOutput=False)
    wl3 = nc.declare_dram_parameter("wl3", [3, 128, 12], F32, isOutput=False)
    out = nc.declare_dram_parameter("out", [12, E_SHARD], F32, isOutput=True)

    with tile.TileContext(nc) as tc:
        with (
            tc.tile_pool(name="const", bufs=1) as cp,
            tc.tile_pool(name="sb", bufs=2) as sb,
            tc.tile_pool(name="xc", bufs=2) as xc,
            tc.tile_pool(name="ps", bufs=2, space="PSUM") as ps,
            tc.tile_pool(name="ps2", bufs=2, space="PSUM") as ps2,
        ):
            # constants (load once, f32r via SWDGE cast where matmul operand)
            w1_t = cp.tile([3, 3, 64], F32R, name="w1_t")
            nc.gpsimd.dma_start(out=w1_t[:], in_=wl1[:].rearrange("n k m -> k n m"))
            w1ab_t = cp.tile([3, 2, 128], F32R, name="w1ab_t")
            nc.gpsimd.dma_start(out=w1ab_t[:], in_=wl1ab[:].rearrange("n k m -> k n m"))
            b1ab_t = cp.tile([128, 1], F32, name="b1ab_t")
            nc.sync.dma_start(out=b1ab_t[:], in_=b1ab[:])
            b1abe_t = cp.tile([128, 1], F32, name="b1abe_t")
            nc.sync.dma_start(out=b1abe_t[:], in_=b1abe[:])
            w2_t = cp.tile([128, 3, 128], F32R, name="w2_t")
            nc.gpsimd.dma_start(out=w2_t[:], in_=wl2[:].rearrange("n k m -> k n m"))
            w3_t = cp.tile([128, 3, 12], F32R, name="w3_t")
            nc.gpsimd.dma_start(out=w3_t[:], in_=wl3[:].rearrange("n k m -> k n m"))
            b1n_t = cp.tile([64, 3, 1], F32, name="b1n_t")
            nc.sync.dma_start(out=b1n_t[:], in_=b1n[:].rearrange("n k o -> k n o"))
            b1ne_t = cp.tile([64, 3, 1], F32, name="b1ne_t")
            nc.sync.dma_start(out=b1ne_t[:], in_=b1ne[:].rearrange("n k o -> k n o"))
            b2r_t = cp.tile([64, 3, 1], F32, name="b2r_t")
            nc.sync.dma_start(out=b2r_t[:], in_=b2r[:].rearrange("n k o -> k n o"))

            for bi in range(N_BATCH):
                if bi % XCHUNK == 0:
                    xck = xc.tile([3, XCHUNK * B], F32R, name="xck", tag="xck")
                    lo = bi * B
                    hi = min(E_SHARD, lo + XCHUNK * B)
                    nc.gpsimd.dma_start(out=xck[:, : hi - lo], in_=x1[:, lo:hi])
                xs = xck[:, (bi % XCHUNK) * B : (bi % XCHUNK) * B + B]

                # ---- L1: nets A+B packed in one [128, B] psum (2 zero-col
                # accumulating f32r MMs at base 0); net C separate [64, B].
                zab = ps.tile([128, B], F32, space="PSUM", name="zab", tag="z1")
                nc.tensor.matmul(out=zab[:], lhsT=w1ab_t[:, 0, :], rhs=xs,
                                 start=True, stop=False)
                nc.tensor.matmul(out=zab[:], lhsT=w1ab_t[:, 1, :], rhs=xs,
                                 start=False, stop=True)
                zc = ps.tile([64, B], F32, space="PSUM", name="zc", tag="z1c")
                nc.tensor.matmul(out=zc[:], lhsT=w1_t[:, 2, :], rhs=xs,
                                 start=True, stop=True)
                h1ab = sb.tile([128, B], F32, name="h1ab", tag="h1ab")
                nc.scalar.activation(h1ab[:], zab[:], act, bias=b1ab_t[:])
                h1abe = sb.tile([128, B], F32, name="h1abe", tag="h1abe")
                nc.scalar.activation(h1abe[:], zab[:], act, bias=b1abe_t[:])
                h1c = sb.tile([64, B], F32, name="h1c", tag="h1c")
                nc.scalar.activation(h1c[:], zc[:], act, bias=b1n_t[:, 2, :])
                h1ce = sb.tile([64, B], F32, name="h1ce", tag="h1ce")
                nc.scalar.activation(h1ce[:], zc[:], act, bias=b1ne_t[:, 2, :])
                # net A: deltas rows 0-63 (aligned sub), bases 64-127 (shifted copy)
                l2sA = sb.tile([128, B], F32R, name="l2sA", tag="l2sA")
                nc.vector.tensor_tensor(out=l2sA[0:64, :], in0=h1abe[0:64, :],
                                        in1=h1ab[0:64, :], op=mybir.AluOpType.subtract)
                nc.vector.tensor_copy(out=l2sA[64:128, :], in_=h1ab[0:64, :])
                # net B: bases rows 0-63 (shifted copy), deltas 64-127 (aligned sub)
                l2sB = sb.tile([128, B], F32R, name="l2sB", tag="l2sB")
                nc.vector.tensor_copy(out=l2sB[0:64, :], in_=h1ab[64:128, :])
                nc.vector.tensor_tensor(out=l2sB[64:128, :], in0=h1abe[64:128, :],
                                        in1=h1ab[64:128, :], op=mybir.AluOpType.subtract)
                # net C: as before
                l2sC = sb.tile([128, B], F32R, name="l2sC", tag="l2sC")
                nc.vector.tensor_tensor(out=l2sC[0:64, :], in0=h1ce[:],
                                        in1=h1c[:], op=mybir.AluOpType.subtract)
                nc.vector.tensor_copy(out=l2sC[64:128, :], in_=h1c[:])
                l2srcs = [l2sA, l2sB, l2sC]

                l3srcs = []
                for g, l2s in enumerate(l2srcs):
                    z2 = ps2.tile([128, B], F32, space="PSUM", name=f"z2_{g}", tag="z2")
                    nc.tensor.matmul(out=z2[:], lhsT=w2_t[:, g, :], rhs=l2s[:],
                                     start=True, stop=True)
                    # z2 layout: rows 0-63 = bases (z2_i, z2_j), 64-127 = dz2
                    dz2 = sb.tile([64, B], F32, name=f"dz2_{g}", tag="dz2")
                    nc.vector.tensor_copy(out=dz2[:], in_=z2[64:128, :])
                    u2 = sb.tile([64, B], F32, name=f"u2_{g}", tag="u2")
                    nc.vector.tensor_tensor(out=u2[:], in0=z2[0:64, :],
                                            in1=dz2[:], op=mybir.AluOpType.add)
                    h2 = sb.tile([64, B], F32, name=f"h2_{g}", tag="h2")
                    nc.scalar.activation(h2[:], z2[0:64, :], act, bias=b2r_t[:, g, :])
                    h2e = sb.tile([64, B], F32, name=f"h2e_{g}", tag="h2e")
                    nc.scalar.activation(h2e[:], u2[:], act, bias=b2r_t[:, g, :])
                    l3s = sb.tile([128, B], F32R, name=f"l3s_{g}", tag="l3s")
                    nc.vector.tensor_tensor(out=l3s[0:64, :], in0=h2e[:],
                                            in1=h2[:], op=mybir.AluOpType.subtract)
                    nc.vector.tensor_copy(out=l3s[64:128, :], in_=h2[:])
                    l3srcs.append(l3s)

                l3o = ps.tile([12, B], F32, space="PSUM", name="l3o", tag="l3o")
                for g, l3s in enumerate(l3srcs):
                    nc.tensor.matmul(out=l3o[:], lhsT=w3_t[:, g, :], rhs=l3s[:],
                                     start=(g == 0), stop=(g == 2))
                ev = sb.tile([12, B], F32, name="ev", tag="ev")
                nc.vector.tensor_copy(out=ev[:], in_=l3o[:])
                nc.sync.dma_start(out=out[:, bi * B : bi * B + B], in_=ev[:])

    if legalize:
        _legalize_waits(nc)
    return nc


# ------------------------------------------------------------------- host ---
def _silu(x):
    return x / (1.0 + np.exp(-x))


def _softplus(x):
    return np.logaddexp(np.float32(0.0), x).astype(np.float32)


def _np_params(params):
    out = {}
    for k in ("W_mlp", "E_mlp", "A_mlp", "B_mlp", "C_mlp"):
        out[k] = [(np.asarray(W, np.float32), np.asarray(b, np.float32))
                  for W, b in params[k]]
    out["log_k_B"] = float(np.asarray(params["log_k_B"]))
    out["log_m"] = float(np.asarray(params["log_m"]))
    return out


def _host_nodes(S, d, params):
    T_SIGNS = np.array([1.0, -1.0], np.float32)
    N = S.shape[0]
    Vn = (1.0 / d).astype(np.float32)
    EPS = np.float32(0.01)
    S_pert = np.concatenate([S, S + EPS, S, S - EPS], 0)
    V_pert = np.concatenate([Vn, Vn, Vn + EPS, Vn], 0)
    x = np.concatenate([S_pert, V_pert], -1)
    layers = params["E_mlp"]
    n = len(layers)
    for idx, (W, b) in enumerate(layers):
        Wa = np.abs(W)
        if idx == 0:
            Wa = Wa * T_SIGNS
        y = x @ Wa.T + b
        x = _softplus(y) if idx < n - 1 else y
    U, U_Sp, U_Vp, U_Sm = x[:N], x[N:2*N], x[2*N:3*N], x[3*N:]
    T = (U_Sp - U) / EPS
    Pn = -(U_Vp - U) / EPS
    C = T * EPS**2 / (U_Sp - 2*U + U_Sm)
    return U, T, Pn, C


def _wnet_host(layers, r, eps):
    """Exact W-net FD on host: Wk(r±eps), dW_dr."""
    (W1, b1), (W2, b2), (W3, b3) = layers

    def mlp(xr):
        h = _silu(np.outer(xr, W1[:, 0]) + b1)
        h = _silu(h @ W2.T + b2)
        return h @ W3.T + b3

    sm = (r - eps) / H
    sp = (r + eps) / H
    om = mlp(sm)[:, 0]
    op = mlp(sp)[:, 0]
    Wk_m = np.exp(om) * (1 - sm**2)
    Wk_p = np.exp(op) * (1 - sp**2)
    return (Wk_p - Wk_m) / (2 * eps * r)


def _pack_weights(params):
    """Device constant arrays. K-layouts: deltas rows 0-63, bases 64-127."""
    packs = {}
    nets = [params["A_mlp"], params["B_mlp"], params["C_mlp"]]
    wl1 = np.zeros((3, 3, 64), np.float32)
    b1n = np.zeros((3, 64, 1), np.float32)
    b1ne = np.zeros((3, 64, 1), np.float32)
    for g, net in enumerate(nets):
        (W1, b1), _, _ = net
        wl1[g, 0, 0:32] = W1[:, 0] / H
        wl1[g, 1, 0:32] = W1[:, 1]
        wl1[g, 0, 32:64] = W1[:, 0] / H
        wl1[g, 2, 32:64] = W1[:, 1]
        b1n[g, :, 0] = np.concatenate([b1, b1])
        shift = EPS_T * W1[:, 1]
        b1ne[g, :, 0] = np.concatenate([b1 + shift, b1 + shift])

    # L2 lhsT [K=128, M=128]: K rows 0-63 = d1(i,j), 64-127 = h1(i,j);
    # M cols: [z2_i, z2_j, dz2_i, dz2_j]
    wl2 = np.zeros((3, 128, 128), np.float32)
    b2r = np.zeros((3, 64, 1), np.float32)
    for g, net in enumerate(nets):
        _, (W2, b2), _ = net
        if g == 1:   # net B: bases rows 0-63, deltas 64-127
            kb, kd = 0, 64
        else:
            kb, kd = 64, 0
        wl2[g, kb:kb+32, 0:32] = W2.T        # z2_i <- h1_i
        wl2[g, kb+32:kb+64, 32:64] = W2.T    # z2_j <- h1_j
        wl2[g, kd:kd+32, 64:96] = W2.T       # dz2_i <- d1_i
        wl2[g, kd+32:kd+64, 96:128] = W2.T   # dz2_j <- d1_j
        b2r[g, :, 0] = np.concatenate([b2, b2])

    # L3 lhsT [K=128, M=12]: K rows 0-63 = d2(i,j), 64-127 = h2(i,j)
    wl3 = np.zeros((3, 128, 12), np.float32)
    for g, net in enumerate(nets):
        _, _, (W3, b3) = net
        w3 = W3[0]
        c = 4 * g
        wl3[g, 64:96, c + 0] = w3             # A_i
        wl3[g, 96:128, c + 1] = w3            # A_j
        wl3[g, 0:32, c + 2] = w3 / EPS_T      # DA_i
        wl3[g, 32:64, c + 3] = w3 / EPS_T     # DA_j
    wl1ab = np.zeros((2, 3, 128), np.float32)
    b1ab = np.zeros((128, 1), np.float32)
    b1abe = np.zeros((128, 1), np.float32)
    for g in range(2):
        (W1, b1), _, _ = nets[g]
        base = 64 * g
        wl1ab[g, 0, base:base+32] = W1[:, 0] / H
        wl1ab[g, 1, base:base+32] = W1[:, 1]
        wl1ab[g, 0, base+32:base+64] = W1[:, 0] / H
        wl1ab[g, 2, base+32:base+64] = W1[:, 1]
        b1ab[base:base+64, 0] = np.concatenate([b1, b1])
        sh = EPS_T * W1[:, 1]
        b1abe[base:base+64, 0] = np.concatenate([b1 + sh, b1 + sh])
    packs.update(wl1=wl1, b1n=b1n, b1ne=b1ne, wl2=wl2, b2r=b2r, wl3=wl3,
                 wl1ab=wl1ab, b1ab=b1ab, b1abe=b1abe)
    return packs


_CACHED = {}


def kernel(v, edge_index, r_ij, S, d, dW, dV, params):
    v = np.asarray(v, np.float32)
    edge_index = np.asarray(edge_index)
    r_ij = np.asarray(r_ij, np.float32)
    S = np.asarray(S, np.float32)
    d_ = np.asarray(d, np.float32)
    dW = np.asarray(dW, np.float32)
    dV = np.asarray(dV, np.float32)
    params = _np_params(params)
    N = v.shape[0]
    E = r_ij.shape[0]
    i_idx = np.asarray(edge_index[0]).astype(np.int64)
    j_idx = np.asarray(edge_index[1]).astype(np.int64)

    k_B = np.float32(math.exp(params["log_k_B"]))
    m = np.float32(math.exp(params["log_m"]))
    sdt = np.float32(1.0 / math.sqrt(DT))
    sq = np.float32(math.sqrt(2 * k_B) * sdt)

    U, T, Pn, C = _host_nodes(S, d_, params)
    T1, P1, C1 = T[:, 0], Pn[:, 0], C[:, 0]

    r_norm = np.sqrt((r_ij ** 2).sum(-1)).astype(np.float32)
    t_i = T1[i_idx].astype(np.float32)
    t_j = T1[j_idx].astype(np.float32)

    # ---- device: 12 MLP channels per edge
    if "nc" not in _CACHED:
        _CACHED["nc"] = build_program()
    nc = _CACHED["nc"]
    packs = _pack_weights(params)

    ES = E_SHARD
    in_maps = []
    for c in range(N_CORES):
        lo = c * (E // N_CORES)
        hi = (c + 1) * (E // N_CORES)
        x1 = np.zeros((3, ES), np.float32)
        n = hi - lo
        x1[0, :n] = r_norm[lo:hi]
        x1[1, :n] = t_i[lo:hi]
        x1[2, :n] = t_j[lo:hi]
        if n < ES:  # pad with edge 0 copies (harmless values)
            x1[0, n:] = r_norm[lo]
            x1[1, n:] = t_i[lo]
            x1[2, n:] = t_j[lo]
        im = dict(packs)
        im["x1"] = x1
        in_maps.append(im)

    res = run_bass_kernel_spmd(nc, in_maps, list(range(N_CORES)))
    outs = [res.results[c]["out"] for c in range(N_CORES)]
    mlp = np.concatenate([o[:, : E // N_CORES] for o in outs], axis=1)  # [12, E]

    A_i, A_j, DA_i, DA_j = mlp[0], mlp[1], mlp[2], mlp[3]
    B_i, B_j, DB_i, DB_j = mlp[4], mlp[5], mlp[6], mlp[7]
    Cc_i, Cc_j, DC_i, DC_j = mlp[8], mlp[9], mlp[10], mlp[11]
    b3A = np.float32(params["A_mlp"][2][1][0])
    b3B = np.float32(params["B_mlp"][2][1][0])
    b3C = np.float32(params["C_mlp"][2][1][0])
    A_i = A_i + b3A; A_j = A_j + b3A
    B_i = B_i + b3B; B_j = B_j + b3B
    Cc_i = Cc_i + b3C; Cc_j = Cc_j + b3C

    # ---- host: per-edge channels
    e_ij = r_ij / (r_norm[:, None] + 1e-8)
    v_ij = v[i_idx] - v[j_idx]
    ev = (e_ij * v_ij).sum(-1).astype(np.float32)
    vv = (v_ij * v_ij).sum(-1).astype(np.float32)
    tr = np.trace(dW, axis1=1, axis2=2).astype(np.float32)
    dW_bar = (0.5 * (dW + np.swapaxes(dW, 1, 2))
              - np.eye(3, dtype=np.float32)[None] * tr[:, None, None] / D)
    dWe = np.einsum('eab,eb->ea', dW_bar, e_ij).astype(np.float32)
    Pd = (P1[i_idx] / d_[i_idx, 0] ** 2 + P1[j_idx] / d_[j_idx, 0] ** 2).astype(np.float32)
    u_i = (1.0 / (C1[i_idx] * T1[i_idx])).astype(np.float32)
    u_j = (1.0 / (C1[j_idx] * T1[j_idx])).astype(np.float32)
    ci = (1.0 / C1[i_idx]).astype(np.float32)
    cj = (1.0 / C1[j_idx]).astype(np.float32)
    Ts = (1.0 / T1[i_idx] + 1.0 / T1[j_idx]).astype(np.float32)
    Td = (1.0 / T1[i_idx] - 1.0 / T1[j_idx]).astype(np.float32)
    dv = dV[:, 0]

    dW_dr = _wnet_host(params["W_mlp"], r_norm, EPS_T)
    grad_W = dW_dr[:, None] * (e_ij * (r_norm[:, None] + 1e-8))
    termPd = Pd[:, None] * grad_W

    A_ij = A_i * A_j; B_ij = B_i * B_j; C_ij = Cc_i * Cc_j
    gA_i = 2 * A_ij * A_j * DA_i; gA_j = 2 * A_ij * A_i * DA_j
    gB_i = 2 * B_ij * B_j * DB_i; gB_j = 2 * B_ij * B_i * DB_j
    gC_i = 2 * C_ij * Cc_j * DC_i; gC_j = 2 * C_ij * Cc_i * DC_j
    A2 = A_ij ** 2; B2 = B_ij ** 2
    auxMSV = 0.5 * A2[:, None] * v_ij + (0.5 * A2 + (B2 - A2) / D)[:, None] * ev[:, None] * e_ij
    termMSV = Ts[:, None] * auxMSV
    tv = -(u_i + u_j)[:, None] * auxMSV
    gi_v = ((gA_i / 2)[:, None] * v_ij
            + (gA_i / 2 + (gB_i - gA_i) / D)[:, None] * ev[:, None] * e_ij) * ci[:, None]
    gj_v = ((gA_j / 2)[:, None] * v_ij
            + (gA_j / 2 + (gB_j - gA_j) / D)[:, None] * ev[:, None] * e_ij) * cj[:, None]
    q = tv + gi_v + gj_v
    noise = A_ij[:, None] * dWe + (B_ij * tr / D)[:, None] * e_ij
    alpha = (-termPd - 0.5 * termMSV - 0.5 * k_B * q) / m + (np.sqrt(2 * k_B) / m * sdt) * noise
    tn = -0.5 * (noise * v_ij).sum(-1)
    tC = C_ij * dv
    auxMSS = (A2 / 2 * vv + (A2 / 2 + (B2 - A2) / D) * ev ** 2) / 4
    C2s = C_ij ** 2
    mS_i = Ts * auxMSS + Td * C2s
    mS_j = Ts * auxMSS - Td * C2s
    s1_i = -(2 * u_i + u_j) * auxMSS
    s1_j = -(2 * u_j + u_i) * auxMSS
    hi = (gA_i / 2 * vv + (gA_i / 2 + (gB_i - gA_i) / D) * ev ** 2) * ci / 4
    hj = (gA_j / 2 * vv + (gA_j / 2 + (gB_j - gA_j) / D) * ev ** 2) * cj / 4
    h_mix = (gA_i / 2 * vv + (gA_j / 2 + (gB_i - gA_i) / D) * ev ** 2) * ci / 4
    s4_i = -(2 * u_i - u_j) * C2s
    s4_j = -(2 * u_j - u_i) * C2s
    p5 = gC_i * ci - gC_j * cj
    tmv = -((D + 1) * A2 / 2 + (B2 - A2) / D)
    b_i = mS_i + k_B * (s1_i + hi + hj + s4_i + p5 + tmv / m) + sq * (tn + tC)
    b_j = mS_j + k_B * (s1_j + hj + h_mix + s4_j - p5 + tmv / m) + sq * (tn - tC)

    # ---- segment sum (host)
    ACC = np.zeros((N, 4), np.float32)
    for c_ in range(3):
        ACC[:, c_] += np.bincount(i_idx, weights=alpha[:, c_], minlength=N).astype(np.float32)
        ACC[:, c_] -= np.bincount(j_idx, weights=alpha[:, c_], minlength=N).astype(np.float32)
    ACC[:, 3] += np.bincount(i_idx, weights=b_i, minlength=N).astype(np.float32)
    ACC[:, 3] += np.bincount(j_idx, weights=b_j, minlength=N).astype(np.float32)

    dvdt = ACC[:, 0:3]
    dSdt = (ACC[:, 3] / T1)[:, None].astype(np.float32)
    E_out = (U + 0.5 * m * (v * v).sum(-1, keepdims=True)).astype(np.float32)
    return dvdt, dSdt, E_out
